# revision 6
# baseline (speedup 1.0000x reference)
"""Trainium2 Bass kernel for nn_Attention_Module (dense_transformer).

Data-parallel over batch: B=64 split across 8 NeuronCores (8 per core).
Per core, activations are channel-major [C, tokens] with the 8 local
batches' 320 tokens reordered into a z-block (8*64=512 template tokens)
followed by an x-block (8*256=2048 search tokens): 5 token-tiles of 512.

v2: bf16 end-to-end (weights, activations, transfers; fp32 PSUM
accumulate), restructured attention (G^T with additive -100 off-diag
head mask -> single exp per (branch, group), softmax denominator via a
1-column matmul, scales via exp(-0.5 ln x) so the scalar engine stays
on one activation table), deliberate engine balancing (Act/DVE/Pool),
merged DMA descriptors.

Self-contained: only imports infra from /opt/trn_rl_repo.
"""
import sys

sys.path.insert(0, "/opt/trn_rl_repo")

from contextlib import ExitStack

import numpy as np

import concourse.bacc as bacc
import concourse.tile as tile
from concourse import mybir
F32 = mybir.dt.float32
BF = mybir.dt.bfloat16
AF = mybir.ActivationFunctionType
OP = mybir.AluOpType
AX = mybir.AxisListType

B_LOC = 8          # batches per core
DIM = 512
HID = 256
HEADS = 8
NZ, NX = 64, 256   # template / search tokens per batch
NTOK = NZ + NX     # 320
T = B_LOC * NTOK   # 2560 reordered tokens per core
NT = 5             # token tiles of 512
EPS_LN = 1e-5
TINY = 1e-24       # guards ln of exact-zero row norms
MASK = -100.0      # additive off-diagonal head mask (pre exp-scale)


def _bbs(j):
    """Branch segments inside token-tile j: list of (batch, col_off, width).

    Tile 0 is the z-block (8 branches of 64), tiles 1..4 hold two x-branches
    of 256 tokens each (batches 2j-2 and 2j-1).
    """
    if j == 0:
        return [(b, 64 * b, 64) for b in range(B_LOC)]
    return [(2 * (j - 1), 0, 256), (2 * j - 1, 256, 256)]


def build_nc():
    nc = bacc.Bacc("TRN2", target_bir_lowering=False, debug=False,
                   num_devices=8)

    # ---- DRAM I/O (per-core shapes) ----
    x1_e = nc.declare_dram_parameter("x1", [B_LOC, DIM, NTOK], BF, isOutput=False)
    x2_e = nc.declare_dram_parameter("x2", [B_LOC, DIM, NTOK], BF, isOutput=False)
    out_e = nc.declare_dram_parameter("out", [B_LOC, DIM, NTOK], BF, isOutput=True)
    wlin_e = nc.declare_dram_parameter("W_lin", [DIM, 2 * DIM], BF, isOutput=False)
    wdown_e = nc.declare_dram_parameter("W_down", [DIM, HID], BF, isOutput=False)
    wup_e = nc.declare_dram_parameter("W_up", [HID, DIM], BF, isOutput=False)
    wq_e = nc.declare_dram_parameter("WqT", [HID, HID], BF, isOutput=False)
    wk_e = nc.declare_dram_parameter("WkT", [HID, HID], BF, isOutput=False)
    wv_e = nc.declare_dram_parameter("WvT", [HID, HID], BF, isOutput=False)
    wo_e = nc.declare_dram_parameter("WoT", [HID, HID], BF, isOutput=False)
    wend_e = nc.declare_dram_parameter("W_end", [DIM, DIM], BF, isOutput=False)
    blin_e = nc.declare_dram_parameter("b_lin", [2 * DIM], F32, isOutput=False)
    bdown_e = nc.declare_dram_parameter("b_down", [HID], F32, isOutput=False)
    bup_e = nc.declare_dram_parameter("b_up", [DIM], F32, isOutput=False)
    bend_e = nc.declare_dram_parameter("b_end", [DIM], F32, isOutput=False)
    gneg_e = nc.declare_dram_parameter("gneg", [DIM], F32, isOutput=False)
    beta_e = nc.declare_dram_parameter("beta", [DIM], F32, isOutput=False)
    lntemp_e = nc.declare_dram_parameter("lntemp_col", [128, 2], F32, isOutput=False)
    ident_e = nc.declare_dram_parameter("ident_in", [128, 128], BF, isOutput=False)

    # merged-descriptor views: channel chunks (kt|m) to partitions
    x1r = x1_e.rearrange("b (kt p) t -> p kt b t", p=128)
    x2r = x2_e.rearrange("b (kt p) t -> p kt b t", p=128)
    outr = out_e.rearrange("b (m p) t -> p m b t", p=128)

    with tile.TileContext(nc) as tc, ExitStack() as ctx:
        wts = ctx.enter_context(tc.tile_pool(name="wts", bufs=1))
        xload = ctx.enter_context(tc.tile_pool(name="xload", bufs=3))
        u1p = ctx.enter_context(tc.tile_pool(name="u1p", bufs=1))
        rp = ctx.enter_context(tc.tile_pool(name="rp", bufs=2))
        u2p = ctx.enter_context(tc.tile_pool(name="u2p", bufs=1))
        ap_ = ctx.enter_context(tc.tile_pool(name="ap", bufs=2))
        bqp = ctx.enter_context(tc.tile_pool(name="bqp", bufs=1))
        qsp = ctx.enter_context(tc.tile_pool(name="qsp", bufs=2))
        nrmp = ctx.enter_context(tc.tile_pool(name="nrmp", bufs=2))
        qtp = ctx.enter_context(tc.tile_pool(name="qtp", bufs=2))
        escp = ctx.enter_context(tc.tile_pool(name="escp", bufs=2))
        avp = ctx.enter_context(tc.tile_pool(name="avp", bufs=2))
        o1p = ctx.enter_context(tc.tile_pool(name="o1p", bufs=1))
        yp = ctx.enter_context(tc.tile_pool(name="yp", bufs=1))
        prep = ctx.enter_context(tc.tile_pool(name="prep", bufs=1))
        statp = ctx.enter_context(tc.tile_pool(name="statp", bufs=1))
        outp = ctx.enter_context(tc.tile_pool(name="outp", bufs=2))
        scr = ctx.enter_context(tc.tile_pool(name="scr", bufs=2))
        # PSUM (8 banks, bank-granular): ps ring 3 (big-mm staging +
        # transposes), s1+s2+pav 3, gps-ring 2 (gps0/gps1/scol share a
        # 2-slot ring; scol may alias a gps slot whose data is already
        # consumed by its mask-add).
        ps = ctx.enter_context(tc.tile_pool(name="ps", bufs=3, space="PSUM"))
        psx = ctx.enter_context(tc.tile_pool(name="psx", bufs=1, space="PSUM"))
        gpool = ctx.enter_context(tc.tile_pool(name="gpool", bufs=2, space="PSUM"))

        # ---- weights / constants to SBUF (DMAs issued later, in priority
        # order, via load_weights()) ----
        wlin_sb = wts.tile([128, 4, 2 * DIM], BF)
        wdown_sb = wts.tile([128, 4, HID], BF)
        wup_sb = wts.tile([128, 2, DIM], BF)
        wq_sb = wts.tile([128, 2, HID], BF)
        wk_sb = wts.tile([128, 2, HID], BF)
        wv_sb = wts.tile([128, 2, HID], BF)
        wo_sb = wts.tile([128, 2, HID], BF)
        wend_sb = wts.tile([128, 4, DIM], BF)

        blin_sb = wts.tile([128, 8], F32)
        bdown_sb = wts.tile([128, 2], F32)
        bup_sb = wts.tile([128, 4], F32)
        bend_sb = wts.tile([128, 4], F32)
        gneg_sb = wts.tile([128, 4], F32)
        beta_sb = wts.tile([128, 4], F32)
        lntemp_sb = wts.tile([128, 2], F32)
        ident_sb = wts.tile([128, 128], BF)

        ones_sb = wts.tile([128, 128], BF)
        mask_sb = wts.tile([128, 128], F32)
        bd = wts.tile([128, 4, 128], BF)
        tiny_sb = wts.tile([128, 1], F32)
        epsln_sb = wts.tile([128, 1], F32)

        def load_weights_early():
            nc.sync.dma_start(wlin_sb[:, :, 512:],
                              wlin_e.rearrange("(kt p) m -> p kt m", p=128)[:, :, 512:])
            nc.sync.dma_start(blin_sb[:], blin_e.rearrange("(m p) -> p m", p=128))
            nc.sync.dma_start(wlin_sb[:, :, 0:512],
                              wlin_e.rearrange("(kt p) m -> p kt m", p=128)[:, :, 0:512])
            nc.sync.dma_start(wdown_sb[:], wdown_e.rearrange("(kt p) m -> p kt m", p=128))
            nc.sync.dma_start(bdown_sb[:], bdown_e.rearrange("(m p) -> p m", p=128))
            nc.sync.dma_start(wq_sb[:], wq_e.rearrange("(kt p) m -> p kt m", p=128))
            nc.sync.dma_start(wk_sb[:], wk_e.rearrange("(kt p) m -> p kt m", p=128))
            nc.sync.dma_start(wv_sb[:], wv_e.rearrange("(kt p) m -> p kt m", p=128))
            nc.sync.dma_start(lntemp_sb[:], lntemp_e[:, :])
            nc.sync.dma_start(ident_sb[:], ident_e[:, :])
            nc.gpsimd.memset(tiny_sb[:], TINY)
            nc.gpsimd.memset(epsln_sb[:], EPS_LN)
            nc.gpsimd.memset(ones_sb[:], 1.0)
            nc.gpsimd.memset(mask_sb[:], MASK)
            for h in range(4):
                nc.gpsimd.memset(mask_sb[32 * h:32 * (h + 1), 32 * h:32 * (h + 1)], 0.0)

        def load_weights_late():
            nc.sync.dma_start(wo_sb[:], wo_e.rearrange("(kt p) m -> p kt m", p=128))
            nc.sync.dma_start(wup_sb[:], wup_e.rearrange("(kt p) m -> p kt m", p=128))
            nc.sync.dma_start(wend_sb[:], wend_e.rearrange("(kt p) m -> p kt m", p=128))
            nc.sync.dma_start(bup_sb[:], bup_e.rearrange("(m p) -> p m", p=128))
            nc.sync.dma_start(bend_sb[:], bend_e.rearrange("(m p) -> p m", p=128))
            nc.sync.dma_start(gneg_sb[:], gneg_e.rearrange("(m p) -> p m", p=128))
            nc.sync.dma_start(beta_sb[:], beta_e.rearrange("(m p) -> p m", p=128))

        def emit_loads(j):
            x1t = xload.tile([128, 4, 512], BF, tag="x1t")
            x2t = xload.tile([128, 4, 512], BF, tag="x2t")
            for tsr, src in ((x1t, x1r), (x2t, x2r)):
                for kt in range(4):
                    if j == 0:
                        nc.sync.dma_start(
                            tsr[:, kt, :].rearrange("p (b t) -> p b t", b=8),
                            src[:, kt, :, 0:64])
                    else:
                        bs = slice(2 * (j - 1), 2 * j)
                        nc.sync.dma_start(
                            tsr[:, kt, :].rearrange("p (b t) -> p b t", b=2),
                            src[:, kt, bs, 64:320])
            return (x1t, x2t)

        def emit_front(j, ld):
            bbs = _bbs(j)
            nb = len(bbs)
            x1t, x2t = ld
            # ---- S1: h1 = relu(W_lin^T X1 + b); u1 kept, r = y1 + u1 ----
            u1 = u1p.tile([128, 4, 512], BF)
            r = rp.tile([128, 4, 512], BF)
            for m in [4, 5, 6, 7, 0, 1, 2, 3]:
                pt = ps.tile([128, 512], F32, tag="ps")
                for kt in range(4):
                    nc.tensor.matmul(pt[:], wlin_sb[:, kt, 128 * m:128 * (m + 1)],
                                     x1t[:, kt, :], start=(kt == 0), stop=(kt == 3))
                if m >= 4:
                    nc.scalar.activation(u1[:, m - 4, :], pt[:], AF.Relu,
                                         bias=blin_sb[:, m:m + 1])
                else:
                    ytmp = scr.tile([128, 512], BF, tag="ytmp")
                    nc.scalar.activation(ytmp[:], pt[:], AF.Relu,
                                         bias=blin_sb[:, m:m + 1])
                    nc.gpsimd.tensor_add(r[:, m, :], ytmp[:], u1[:, m, :])

            # ---- S1b: u2 = relu(W_lin[:,512:]^T X2 + b2) (DVE relu) ----
            u2 = u2p.tile([128, 4, 512], BF)
            for m in range(4):
                pt = ps.tile([128, 512], F32, tag="ps")
                for kt in range(4):
                    nc.tensor.matmul(
                        pt[:], wlin_sb[:, kt, 512 + 128 * m:512 + 128 * (m + 1)],
                        x2t[:, kt, :], start=(kt == 0), stop=(kt == 3))
                nc.vector.tensor_scalar(
                    u2[:, m, :], in0=pt[:], scalar1=blin_sb[:, 4 + m:5 + m],
                    scalar2=0.0, op0=OP.add, op1=OP.max)

            # ---- S2: A = relu(W_down^T u1 + b_down); Bq likewise from u2 ----
            A = ap_.tile([128, 2, 512], BF)
            Bq = bqp.tile([128, 2, 512], BF)
            for (dst, src) in ((A, u1), (Bq, u2)):
                for m in range(2):
                    pt = ps.tile([128, 512], F32, tag="ps")
                    for kt in range(4):
                        nc.tensor.matmul(pt[:],
                                         wdown_sb[:, kt, 128 * m:128 * (m + 1)],
                                         src[:, kt, :],
                                         start=(kt == 0), stop=(kt == 3))
                    nc.scalar.activation(dst[:, m, :], pt[:], AF.Relu,
                                         bias=bdown_sb[:, m:m + 1])

            # ---- S3/S4: q,k,v channel-major; norms over tokens per branch.
            # q is normalized on extraction; k's norm (x temperature) is kept
            # for the exp scale; v copied plain.
            q_sb = qsp.tile([128, 2, 512], BF, tag="q")
            k_sb = qsp.tile([128, 2, 512], BF, tag="k")
            v_sb = qsp.tile([128, 2, 512], BF, tag="v")
            ssq_q = nrmp.tile([128, 2, 8], F32, tag="ssq_q")
            ssq_k = nrmp.tile([128, 2, 8], F32, tag="ssq_k")
            lnt = nrmp.tile([128, 2, 8], F32, tag="lnt")
            rn_q = nrmp.tile([128, 2, 8], F32, tag="rn_q")
            rn_kt = nrmp.tile([128, 2, 8], F32, tag="rn_kt")
            sqscr = scr.tile([128, 2, 512], BF, tag="sqscr")

            q_ps = []
            for g in range(2):
                pt = ps.tile([128, 512], F32, tag="ps")
                for kt in range(2):
                    nc.tensor.matmul(pt[:], wq_sb[:, kt, 128 * g:128 * (g + 1)],
                                     Bq[:, kt, :], start=(kt == 0), stop=(kt == 1))
                q_ps.append(pt)
                for bi, (b, off, w) in enumerate(bbs):
                    nc.scalar.activation(
                        sqscr[:, g, off:off + w], pt[:, off:off + w], AF.Square,
                        accum_out=ssq_q[:, g, bi:bi + 1])
            nc.scalar.activation(lnt[:, :, 0:nb], ssq_q[:, :, 0:nb], AF.Ln,
                                 bias=tiny_sb[:, 0:1])
            for g in range(2):
                nc.scalar.activation(rn_q[:, g, 0:nb], lnt[:, g, 0:nb], AF.Exp,
                                     scale=-0.5)
            for g in range(2):
                for bi, (b, off, w) in enumerate(bbs):
                    nc.scalar.activation(
                        q_sb[:, g, off:off + w], q_ps[g][:, off:off + w],
                        AF.Copy, scale=rn_q[:, g, bi:bi + 1])

            for g in range(2):
                pt = ps.tile([128, 512], F32, tag="ps")
                for kt in range(2):
                    nc.tensor.matmul(pt[:], wk_sb[:, kt, 128 * g:128 * (g + 1)],
                                     A[:, kt, :], start=(kt == 0), stop=(kt == 1))
                for bi, (b, off, w) in enumerate(bbs):
                    nc.scalar.activation(
                        sqscr[:, g, off:off + w], pt[:, off:off + w], AF.Square,
                        accum_out=ssq_k[:, g, bi:bi + 1])
                nc.vector.tensor_copy(k_sb[:, g, :], pt[:])
            nc.scalar.activation(lnt[:, :, 0:nb], ssq_k[:, :, 0:nb], AF.Ln,
                                 bias=tiny_sb[:, 0:1])
            for g in range(2):
                nc.scalar.activation(rn_kt[:, g, 0:nb], lnt[:, g, 0:nb], AF.Exp,
                                     scale=-0.5, bias=lntemp_sb[:, g:g + 1])

            for g in range(2):
                pt = ps.tile([128, 512], F32, tag="ps")
                for kt in range(2):
                    nc.tensor.matmul(pt[:], wv_sb[:, kt, 128 * g:128 * (g + 1)],
                                     A[:, kt, :], start=(kt == 0), stop=(kt == 1))
                nc.vector.tensor_copy(v_sb[:, g, :], pt[:])

            return dict(x1t=x1t, r=r, A=A, q=q_sb, k=k_sb, v=v_sb, rn_kt=rn_kt)

        def emit_back(j, st):
            bbs = _bbs(j)
            nb = len(bbs)
            x1t, r, A = st["x1t"], st["r"], st["A"]
            q_sb, k_sb, v_sb, rn_kt = st["q"], st["k"], st["v"], st["rn_kt"]

            # ---- S5: PE-transpose q̂,k -> token-major qT,kT (bf16) ----
            qT = qtp.tile([128, 4, 256], BF, tag="qT")
            kT = qtp.tile([128, 4, 256], BF, tag="kT")
            for (dst, src) in ((qT, q_sb), (kT, k_sb)):
                for tb in range(4):
                    pt = ps.tile([128, 256], BF, tag="ps")
                    for g in range(2):
                        nc.tensor.matmul(
                            pt[:, 128 * g:128 * (g + 1)],
                            src[:, g, 128 * tb:128 * (tb + 1)], ident_sb[:],
                            is_transpose=True, start=(g == 0), stop=(g == 1))
                    nc.vector.tensor_copy(
                        dst[:, tb, :].bitcast(mybir.dt.uint32),
                        pt[:].bitcast(mybir.dt.uint32))

            # ---- S6-S8 per-branch attention: G^T -> mask-add -> exp into
            # block-diag slot -> S column + AV matmuls -> scaled extract.
            av = avp.tile([128, 2, 512], BF)
            lnS = nrmp.tile([128, 2, 8], F32, tag="lnS")
            R_t = nrmp.tile([128, 2, 8], F32, tag="R_t")
            for bi, (b, off, w) in enumerate(bbs):
                if j == 0:
                    chunks = [(off // 128, off % 128, 64)]
                else:
                    chunks = [(off // 128, 0, 128), (off // 128 + 1, 0, 128)]
                esc = escp.tile([128, 2, 128], F32, tag="esc")
                gps = [gpool.tile([128, 128], F32, tag="gps", name=f"gps{g_}")
                       for g_ in range(2)]
                scol = gpool.tile([128, 2, 1], F32, tag="gps", name="scol")
                for g in range(2):
                    for ci, (tb, tpo, cw) in enumerate(chunks):
                        nc.tensor.matmul(
                            gps[g][:, :],
                            kT[tpo:tpo + cw, tb, 128 * g:128 * (g + 1)],
                            qT[tpo:tpo + cw, tb, 128 * g:128 * (g + 1)],
                            start=(ci == 0), stop=(ci == len(chunks) - 1))
                for g in range(2):
                    bsl = 2 * (bi % 2) + g
                    nc.vector.tensor_add(esc[:, g, :], gps[g][:], mask_sb[:])
                    nc.scalar.activation(bd[:, bsl, :], esc[:, g, :], AF.Exp,
                                         scale=rn_kt[:, g, bi:bi + 1])
                    nc.tensor.matmul(scol[:, g, :], bd[:, bsl, :],
                                     ones_sb[:, 0:1], start=True, stop=True)
                pav = psx.tile([128, 2, w], F32, tag="pav")
                for g in range(2):
                    bsl = 2 * (bi % 2) + g
                    nc.tensor.matmul(pav[:, g, :], bd[:, bsl, :],
                                     v_sb[:, g, off:off + w], start=True, stop=True)
                nc.scalar.activation(lnS[:, :, bi:bi + 1], scol[:, :, :], AF.Ln)
                nc.scalar.activation(R_t[:, :, bi:bi + 1], lnS[:, :, bi:bi + 1],
                                     AF.Exp, scale=-1.0)
                for g in range(2):
                    nc.scalar.activation(
                        av[:, g, off:off + w], pav[:, g, :], AF.Copy,
                        scale=R_t[:, g, bi:bi + 1])

            # ---- S9: o1 = Wo@av + A (res1) ----
            o1 = o1p.tile([128, 2, 512], BF)
            for m in range(2):
                pt = ps.tile([128, 512], F32, tag="ps")
                for kt in range(2):
                    nc.tensor.matmul(pt[:], wo_sb[:, kt, 128 * m:128 * (m + 1)],
                                     av[:, kt, :], start=(kt == 0), stop=(kt == 1))
                nc.vector.tensor_add(o1[:, m, :], pt[:], A[:, m, :])

            # ---- S10: y = W_up^T o1 + b_up + r ----
            y = yp.tile([128, 4, 512], BF)
            for m in range(4):
                pt = ps.tile([128, 512], F32, tag="ps")
                for kt in range(2):
                    nc.tensor.matmul(pt[:], wup_sb[:, kt, 128 * m:128 * (m + 1)],
                                     o1[:, kt, :], start=(kt == 0), stop=(kt == 1))
                nc.vector.scalar_tensor_tensor(
                    y[:, m, :], in0=pt[:], scalar=bup_sb[:, m:m + 1],
                    in1=r[:, m, :], op0=OP.add, op1=OP.add)

            # ---- S11: pre = W_end^T y + b_end + t1 ; LN stats via ones-matmul
            pre = prep.tile([128, 4, 512], BF, tag="pre")
            p2 = prep.tile([128, 4, 512], BF, tag="p2")
            s1ps = psx.tile([128, 512], F32, tag="s1")
            s2ps = psx.tile([128, 512], F32, tag="s2")
            for m in range(4):
                pt = ps.tile([128, 512], F32, tag="ps")
                for kt in range(4):
                    nc.tensor.matmul(pt[:], wend_sb[:, kt, 128 * m:128 * (m + 1)],
                                     y[:, kt, :], start=(kt == 0), stop=(kt == 3))
                nc.vector.scalar_tensor_tensor(
                    pre[:, m, :], in0=pt[:], scalar=bend_sb[:, m:m + 1],
                    in1=x1t[:, m, :], op0=OP.add, op1=OP.add)
                nc.gpsimd.tensor_mul(p2[:, m, :], pre[:, m, :], pre[:, m, :])
                nc.tensor.matmul(s1ps[:], ones_sb[:], pre[:, m, :],
                                 start=(m == 0), stop=(m == 3))
                nc.tensor.matmul(s2ps[:], ones_sb[:], p2[:, m, :],
                                 start=(m == 0), stop=(m == 3))

            # ---- S12: mu / rstd (rows replicated); rstd = exp(-0.5 ln(var+eps))
            mu = statp.tile([128, 512], F32, tag="mu")
            nc.vector.tensor_scalar_mul(mu[:], in0=s1ps[:], scalar1=1.0 / DIM)
            mu2 = statp.tile([128, 512], F32, tag="mu2")
            nc.vector.tensor_mul(mu2[:], mu[:], mu[:])
            var = statp.tile([128, 512], F32, tag="var")
            nc.vector.scalar_tensor_tensor(var[:], in0=s2ps[:], scalar=1.0 / DIM,
                                           in1=mu2[:], op0=OP.mult, op1=OP.subtract)
            lnv = statp.tile([128, 512], F32, tag="lnv")
            nc.scalar.activation(lnv[:], var[:], AF.Ln, bias=epsln_sb[:, 0:1])
            rstd = statp.tile([128, 512], BF, tag="rstd")
            nc.scalar.activation(rstd[:], lnv[:], AF.Exp, scale=-0.5)

            # ---- S13: out = (mu - pre)*rstd*(-gamma) + beta ----
            ot = outp.tile([128, 4, 512], BF)
            for m in range(4):
                d_m = scr.tile([128, 512], BF, tag="d_m")
                nc.vector.scalar_tensor_tensor(
                    d_m[:], in0=s1ps[:], scalar=1.0 / DIM,
                    in1=pre[:, m, :], op0=OP.mult, op1=OP.subtract)
                e_m = scr.tile([128, 512], BF, tag="e_m")
                nc.vector.tensor_mul(e_m[:], d_m[:], rstd[:])
                nc.vector.tensor_scalar(
                    ot[:, m, :], in0=e_m[:], scalar1=gneg_sb[:, m:m + 1],
                    scalar2=beta_sb[:, m:m + 1], op0=OP.mult, op1=OP.add)

            # ---- S14: store (un-reorder tokens) ----
            for m in range(4):
                if j == 0:
                    nc.sync.dma_start(
                        outr[:, m, :, 0:64],
                        ot[:, m, :].rearrange("p (b t) -> p b t", b=8))
                else:
                    bs = slice(2 * (j - 1), 2 * j)
                    nc.sync.dma_start(
                        outr[:, m, bs, 64:320],
                        ot[:, m, :].rearrange("p (b t) -> p b t", b=2))

        prev = None
        order = [1, 2, 0, 3, 4]
        for j in order:
            ld = emit_loads(j)
            if j == order[0]:
                load_weights_early()
            st = emit_front(j, ld)
            if j == order[0]:
                load_weights_late()
            if prev is not None:
                emit_back(prev[0], prev[1])
            prev = (j, st)
        emit_back(prev[0], prev[1])

    nc.compile()
    return nc


# ---------------- host side ----------------
_CACHE = {}


def _get_runner():
    if "runner" in _CACHE:
        return _CACHE["runner"]
    import jax
    from jax.sharding import Mesh, PartitionSpec
    from jax.experimental.shard_map import shard_map
    from concourse.bass2jax import (
        _bass_exec_p, install_neuronx_cc_hook, partition_id_tensor)
    import concourse.mybir as mybir_

    nc = build_nc()
    install_neuronx_cc_hook()
    partition_name = nc.partition_id_tensor.name if nc.partition_id_tensor else None
    in_names, out_names, out_avals, zero_outs = [], [], [], []
    for alloc in nc.m.functions[0].allocations:
        if not isinstance(alloc, mybir_.MemoryLocationSet):
            continue
        name = alloc.memorylocations[0].name
        if alloc.kind == "ExternalInput":
            if name != partition_name:
                in_names.append(name)
        elif alloc.kind == "ExternalOutput":
            out_names.append(name)
            shape = tuple(alloc.tensor_shape)
            dtype = mybir_.dt.np(alloc.dtype)
            out_avals.append(jax.core.ShapedArray(shape, dtype))
            zero_outs.append(np.zeros(shape, dtype))
    n_params, n_outs = len(in_names), len(out_avals)
    all_in = list(in_names) + list(out_names)
    if partition_name is not None:
        all_in.append(partition_name)
    donate = tuple(range(n_params, n_params + n_outs))

    def _body(*args):
        operands = list(args)
        if partition_name is not None:
            operands.append(partition_id_tensor())
        return tuple(_bass_exec_p.bind(
            *operands, out_avals=tuple(out_avals), in_names=tuple(all_in),
            out_names=tuple(out_names), lowering_input_output_aliases=(),
            sim_require_finite=True, sim_require_nnan=True, nc=nc))

    devices = jax.devices()[:8]
    mesh = Mesh(np.asarray(devices), ("core",))
    fn = jax.jit(
        shard_map(_body, mesh=mesh,
                  in_specs=(PartitionSpec("core"),) * (n_params + n_outs),
                  out_specs=(PartitionSpec("core"),) * n_outs,
                  check_rep=False),
        donate_argnums=donate, keep_unused=True)
    _CACHE["runner"] = (fn, in_names, out_names, out_avals, zero_outs)
    return _CACHE["runner"]


def _prep_inputs(inputs):
    import ml_dtypes
    BFNP = ml_dtypes.bfloat16
    f = lambda a: np.ascontiguousarray(np.asarray(a), dtype=np.float32)
    bfc = lambda a: np.ascontiguousarray(np.asarray(a, dtype=np.float32).astype(BFNP))
    x1 = bfc(np.asarray(inputs["x1"], dtype=np.float32).reshape(64, DIM, NTOK))
    x2 = bfc(np.asarray(inputs["x2"], dtype=np.float32).reshape(64, DIM, NTOK))
    temp = f(inputs["temperature"]).reshape(HEADS)
    # lntemp_col[p, g] = ln(temperature[4*g + p//32])
    lntemp = np.log(np.maximum(temp, 1e-30)).astype(np.float32)
    lntemp_col = np.empty((128, 2), np.float32)
    for g in range(2):
        for hh in range(4):
            lntemp_col[32 * hh:32 * (hh + 1), g] = lntemp[4 * g + hh]
    shared = {
        "W_lin": bfc(inputs["W_lin"]), "W_down": bfc(inputs["W_down"]),
        "W_up": bfc(inputs["W_up"]),
        "WqT": bfc(np.asarray(inputs["Wq"], dtype=np.float32).T),
        "WkT": bfc(np.asarray(inputs["Wk"], dtype=np.float32).T),
        "WvT": bfc(np.asarray(inputs["Wv"], dtype=np.float32).T),
        "WoT": bfc(np.asarray(inputs["Wo"], dtype=np.float32).T),
        "W_end": bfc(inputs["W_end"]), "b_lin": f(inputs["b_lin"]),
        "b_down": f(inputs["b_down"]), "b_up": f(inputs["b_up"]),
        "b_end": f(inputs["b_end"]),
        "gneg": -f(inputs["gamma"]),
        "beta": f(inputs["beta"]), "lntemp_col": lntemp_col,
        "ident_in": np.eye(128).astype(BFNP),
    }
    in_maps = []
    for c in range(8):
        m = dict(shared)
        m["x1"] = np.ascontiguousarray(x1[8 * c:8 * (c + 1)])
        m["x2"] = np.ascontiguousarray(x2[8 * c:8 * (c + 1)])
        in_maps.append(m)
    return in_maps


def run_in_maps(in_maps):
    """Run the prebuilt executable on 8 cores; returns per-core out arrays."""
    import jax
    fn, in_names, out_names, out_avals, zero_outs = _get_runner()
    per_core = [[np.asarray(m[name]) for name in in_names] for m in in_maps]
    concat_in = [np.concatenate([per_core[c][i] for c in range(8)], axis=0)
                 for i in range(len(in_names))]
    concat_zeros = [np.zeros((8 * z.shape[0], *z.shape[1:]), z.dtype)
                    for z in zero_outs]
    out = fn(*concat_in, *concat_zeros)
    jax.block_until_ready(out)
    oi = out_names.index("out")
    arr = np.asarray(out[oi]).reshape(8, *out_avals[oi].shape)
    return arr


def kernel(**inputs):
    in_maps = _prep_inputs(inputs)
    arr = run_in_maps(in_maps)  # [8, 8, 512, 320] bf16
    full = arr.astype(np.float32).reshape(64, DIM, NTOK).reshape(64, DIM, 16, 20)
    return full


if __name__ == "__main__":
    rng = np.random.default_rng(0)
    ins = {
        "x1": rng.standard_normal((64, 512, 16, 20), dtype=np.float32),
        "x2": rng.standard_normal((64, 512, 16, 20), dtype=np.float32),
    }
    s = 0.02
    for nm, shape in [("W_lin", (512, 1024)), ("W_down", (512, 256)),
                      ("W_up", (256, 512)), ("Wq", (256, 256)),
                      ("Wk", (256, 256)), ("Wv", (256, 256)),
                      ("Wo", (256, 256)), ("W_end", (512, 512))]:
        ins[nm] = (rng.standard_normal(shape) * s).astype(np.float32)
    for nm, n in [("b_lin", 1024), ("b_down", 256), ("b_up", 512),
                  ("b_end", 512)]:
        ins[nm] = np.zeros(n, np.float32)
    ins["gamma"] = np.ones(512, np.float32)
    ins["beta"] = np.zeros(512, np.float32)
    ins["temperature"] = np.ones((8, 1, 1), np.float32)
    out = kernel(**ins)
    print("kernel ran, out shape", out.shape, "mean", float(np.abs(out).mean()))


# revision 24
# speedup vs baseline: 1.1552x; 1.1552x over previous
"""Trainium2 Bass kernel for nn_Attention_Module (dense_transformer).

Data-parallel over batch: B=64 split across 8 NeuronCores (8 per core).
Per core, activations are channel-major [C, tokens] with the 8 local
batches' 320 tokens reordered into a z-block (8*64=512 template tokens)
followed by an x-block (8*256=2048 search tokens): 5 token-tiles of 512.

v2: bf16 end-to-end (weights, activations, transfers; fp32 PSUM
accumulate), restructured attention (G^T with additive -100 off-diag
head mask -> single exp per (branch, group), softmax denominator via a
1-column matmul, scales via exp(-0.5 ln x) so the scalar engine stays
on one activation table), deliberate engine balancing (Act/DVE/Pool),
merged DMA descriptors.

Self-contained: only imports infra from /opt/trn_rl_repo.
"""
import sys

sys.path.insert(0, "/opt/trn_rl_repo")

from contextlib import ExitStack

import numpy as np

import concourse.bacc as bacc
import concourse.tile as tile
from concourse import mybir
F32 = mybir.dt.float32
BF = mybir.dt.bfloat16
AF = mybir.ActivationFunctionType
OP = mybir.AluOpType
AX = mybir.AxisListType

B_LOC = 8          # batches per core
DIM = 512
HID = 256
HEADS = 8
NZ, NX = 64, 256   # template / search tokens per batch
NTOK = NZ + NX     # 320
T = B_LOC * NTOK   # 2560 reordered tokens per core
NT = 5             # token tiles of 512
EPS_LN = 1e-5
TINY = 1e-24       # guards ln of exact-zero row norms
MASK = -100.0      # additive off-diagonal head mask (pre exp-scale)


def _bbs(j):
    """Branch segments inside token-tile j: list of (batch, col_off, width).

    Tile 0 is the z-block (8 branches of 64), tiles 1..4 hold two x-branches
    of 256 tokens each (batches 2j-2 and 2j-1).
    """
    if j == 0:
        return [(b, 64 * b, 64) for b in range(B_LOC)]
    return [(2 * (j - 1), 0, 256), (2 * j - 1, 256, 256)]


def build_nc():
    nc = bacc.Bacc("TRN2", target_bir_lowering=False, debug=False,
                   num_devices=8)

    # ---- DRAM I/O (per-core shapes) ----
    x1_e = nc.declare_dram_parameter("x1", [B_LOC, DIM, NTOK], BF, isOutput=False)
    x2_e = nc.declare_dram_parameter("x2", [B_LOC, DIM, NTOK], BF, isOutput=False)
    out_e = nc.declare_dram_parameter("out", [B_LOC, DIM, NTOK], BF, isOutput=True)
    wlin_e = nc.declare_dram_parameter("W_lin", [DIM, 2 * DIM], BF, isOutput=False)
    wdown_e = nc.declare_dram_parameter("W_down", [DIM, HID], BF, isOutput=False)
    wup_e = nc.declare_dram_parameter("W_up", [HID, DIM], BF, isOutput=False)
    wq_e = nc.declare_dram_parameter("WqT", [HID, HID], BF, isOutput=False)
    wk_e = nc.declare_dram_parameter("WkT", [HID, HID], BF, isOutput=False)
    wv_e = nc.declare_dram_parameter("WvT", [HID, HID], BF, isOutput=False)
    wo_e = nc.declare_dram_parameter("WoT", [HID, HID], BF, isOutput=False)
    wend_e = nc.declare_dram_parameter("W_end", [DIM, DIM], BF, isOutput=False)
    blin_e = nc.declare_dram_parameter("b_lin", [2 * DIM], F32, isOutput=False)
    bdown_e = nc.declare_dram_parameter("b_down", [HID], F32, isOutput=False)
    bup_e = nc.declare_dram_parameter("b_up", [DIM], F32, isOutput=False)
    bend_e = nc.declare_dram_parameter("b_end", [DIM], F32, isOutput=False)
    gneg_e = nc.declare_dram_parameter("gneg", [DIM], F32, isOutput=False)
    beta_e = nc.declare_dram_parameter("beta", [DIM], F32, isOutput=False)
    temp_e = nc.declare_dram_parameter("lntemp_col", [128, 2], F32, isOutput=False)
    ident_e = nc.declare_dram_parameter("ident_in", [128, 128], BF, isOutput=False)

    # merged-descriptor views: channel chunks (kt|m) to partitions
    x1r = x1_e.rearrange("b (kt p) t -> p kt b t", p=128)
    x2r = x2_e.rearrange("b (kt p) t -> p kt b t", p=128)
    outr = out_e.rearrange("b (m p) t -> p m b t", p=128)

    with tile.TileContext(nc) as tc, ExitStack() as ctx:
        wts = ctx.enter_context(tc.tile_pool(name="wts", bufs=1))
        xload = ctx.enter_context(tc.tile_pool(name="xload", bufs=3))
        u1p = ctx.enter_context(tc.tile_pool(name="u1p", bufs=1))
        rp = ctx.enter_context(tc.tile_pool(name="rp", bufs=2))
        u2p = ctx.enter_context(tc.tile_pool(name="u2p", bufs=1))
        ap_ = ctx.enter_context(tc.tile_pool(name="ap", bufs=2))
        bqp = ctx.enter_context(tc.tile_pool(name="bqp", bufs=1))
        qsp = ctx.enter_context(tc.tile_pool(name="qsp", bufs=2))
        nrmp = ctx.enter_context(tc.tile_pool(name="nrmp", bufs=2))
        qtp = ctx.enter_context(tc.tile_pool(name="qtp", bufs=2))
        escp = ctx.enter_context(tc.tile_pool(name="escp", bufs=2))
        avp = ctx.enter_context(tc.tile_pool(name="avp", bufs=2))
        o1p = ctx.enter_context(tc.tile_pool(name="o1p", bufs=1))
        yp = ctx.enter_context(tc.tile_pool(name="yp", bufs=1))
        prep = ctx.enter_context(tc.tile_pool(name="prep", bufs=1))
        statp = ctx.enter_context(tc.tile_pool(name="statp", bufs=1))
        outp = ctx.enter_context(tc.tile_pool(name="outp", bufs=2))
        scr = ctx.enter_context(tc.tile_pool(name="scr", bufs=2))
        # PSUM (8 banks, bank-granular): ps ring 3 (big-mm staging +
        # transposes), s1+s2+pav 3, gps-ring 2 (gps0/gps1/scol share a
        # 2-slot ring; scol may alias a gps slot whose data is already
        # consumed by its mask-add).
        ps = ctx.enter_context(tc.tile_pool(name="ps", bufs=3, space="PSUM"))
        psx = ctx.enter_context(tc.tile_pool(name="psx", bufs=1, space="PSUM"))
        gpool = ctx.enter_context(tc.tile_pool(name="gpool", bufs=2, space="PSUM"))

        # ---- weights / constants to SBUF (DMAs issued later, in priority
        # order, via load_weights()) ----
        wlin_sb = wts.tile([128, 4, 2 * DIM], BF)
        wdown_sb = wts.tile([128, 4, HID], BF)
        wup_sb = wts.tile([128, 2, DIM], BF)
        wq_sb = wts.tile([128, 2, HID], BF)
        wk_sb = wts.tile([128, 2, HID], BF)
        wv_sb = wts.tile([128, 2, HID], BF)
        wo_sb = wts.tile([128, 2, HID], BF)
        wend_sb = wts.tile([128, 4, DIM], BF)

        blin_sb = wts.tile([128, 8], F32)
        bdown_sb = wts.tile([128, 2], F32)
        bup_sb = wts.tile([128, 4], F32)
        bend_sb = wts.tile([128, 4], F32)
        gneg_sb = wts.tile([128, 4], F32)
        beta_sb = wts.tile([128, 4], F32)
        lntemp_sb = wts.tile([128, 2], F32)
        ident_sb = wts.tile([128, 128], BF)

        ones_sb = wts.tile([128, 128], BF)
        mask_sb = wts.tile([128, 128], F32)
        bd = wts.tile([128, 4, 128], BF)
        tiny_sb = wts.tile([128, 1], F32)
        epsln_sb = wts.tile([128, 1], F32)

        def load_weights_early():
            # split issue cost across the two HWDGE queues (SP + Act)
            nc.sync.dma_start(wlin_sb[:, :, 512:],
                                wlin_e.rearrange("(kt p) m -> p kt m", p=128)[:, :, 512:])
            nc.sync.dma_start(blin_sb[:], blin_e.rearrange("(m p) -> p m", p=128))
            nc.sync.dma_start(wlin_sb[:, :, 0:512],
                                wlin_e.rearrange("(kt p) m -> p kt m", p=128)[:, :, 0:512])
            nc.sync.dma_start(wdown_sb[:], wdown_e.rearrange("(kt p) m -> p kt m", p=128))
            nc.sync.dma_start(bdown_sb[:], bdown_e.rearrange("(m p) -> p m", p=128))
            nc.sync.dma_start(wq_sb[:], wq_e.rearrange("(kt p) m -> p kt m", p=128))
            nc.sync.dma_start(wk_sb[:], wk_e.rearrange("(kt p) m -> p kt m", p=128))
            nc.sync.dma_start(wv_sb[:], wv_e.rearrange("(kt p) m -> p kt m", p=128))
            nc.sync.dma_start(lntemp_sb[:], temp_e[:, :])
            nc.sync.dma_start(ident_sb[:], ident_e[:, :])
            nc.gpsimd.memset(tiny_sb[:], TINY)
            nc.gpsimd.memset(epsln_sb[:], EPS_LN)
            nc.gpsimd.memset(ones_sb[:], 1.0)
            nc.gpsimd.memset(mask_sb[:], MASK)
            for h in range(4):
                nc.gpsimd.memset(mask_sb[32 * h:32 * (h + 1), 32 * h:32 * (h + 1)], 0.0)

        def load_weights_late():
            nc.sync.dma_start(wo_sb[:], wo_e.rearrange("(kt p) m -> p kt m", p=128))
            nc.sync.dma_start(wup_sb[:], wup_e.rearrange("(kt p) m -> p kt m", p=128))
            nc.sync.dma_start(wend_sb[:], wend_e.rearrange("(kt p) m -> p kt m", p=128))
            nc.sync.dma_start(bup_sb[:], bup_e.rearrange("(m p) -> p m", p=128))
            nc.sync.dma_start(bend_sb[:], bend_e.rearrange("(m p) -> p m", p=128))
            nc.sync.dma_start(gneg_sb[:], gneg_e.rearrange("(m p) -> p m", p=128))
            nc.sync.dma_start(beta_sb[:], beta_e.rearrange("(m p) -> p m", p=128))

        def emit_loads(j):
            x1t = xload.tile([128, 4, 512], BF, tag="x1t")
            x2t = xload.tile([128, 4, 512], BF, tag="x2t")
            for tsr, src in ((x1t, x1r), (x2t, x2r)):
                for kt in range(4):
                    if j == 0:
                        nc.sync.dma_start(
                            tsr[:, kt, :].rearrange("p (b t) -> p b t", b=8),
                            src[:, kt, :, 0:64])
                    else:
                        bs = slice(2 * (j - 1), 2 * j)
                        nc.sync.dma_start(
                            tsr[:, kt, :].rearrange("p (b t) -> p b t", b=2),
                            src[:, kt, bs, 64:320])
            return (x1t, x2t)

        def emit_front(j, ld):
            bbs = _bbs(j)
            nb = len(bbs)
            x1t, x2t = ld
            # ---- S1: h1 = relu(W_lin^T X1 + b); u1 kept, r = y1 + u1 ----
            u1 = u1p.tile([128, 4, 512], BF)
            r = rp.tile([128, 4, 512], BF)
            for m in [4, 5, 6, 7, 0, 1, 2, 3]:
                pt = ps.tile([128, 512], F32, tag="ps")
                for kt in range(4):
                    nc.tensor.matmul(pt[:], wlin_sb[:, kt, 128 * m:128 * (m + 1)],
                                     x1t[:, kt, :], start=(kt == 0), stop=(kt == 3))
                if m >= 4:
                    nc.scalar.activation(u1[:, m - 4, :], pt[:], AF.Relu,
                                         bias=blin_sb[:, m:m + 1])
                else:
                    ytmp = scr.tile([128, 512], BF, tag="ytmp")
                    nc.scalar.activation(ytmp[:], pt[:], AF.Relu,
                                         bias=blin_sb[:, m:m + 1])
                    nc.gpsimd.tensor_add(r[:, m, :], ytmp[:], u1[:, m, :])

            # ---- S1b: u2 = relu(W_lin[:,512:]^T X2 + b2) ----
            u2 = u2p.tile([128, 4, 512], BF)
            for m in range(4):
                pt = ps.tile([128, 512], F32, tag="ps")
                for kt in range(4):
                    nc.tensor.matmul(
                        pt[:], wlin_sb[:, kt, 512 + 128 * m:512 + 128 * (m + 1)],
                        x2t[:, kt, :], start=(kt == 0), stop=(kt == 3))
                nc.scalar.activation(u2[:, m, :], pt[:], AF.Relu,
                                     bias=blin_sb[:, 4 + m:5 + m])

            # ---- S2: A = relu(W_down^T u1 + b_down); Bq likewise from u2 ----
            A = ap_.tile([128, 2, 512], BF)
            Bq = bqp.tile([128, 2, 512], BF)
            for (dst, src) in ((A, u1), (Bq, u2)):
                for m in range(2):
                    pt = ps.tile([128, 512], F32, tag="ps")
                    for kt in range(4):
                        nc.tensor.matmul(pt[:],
                                         wdown_sb[:, kt, 128 * m:128 * (m + 1)],
                                         src[:, kt, :],
                                         start=(kt == 0), stop=(kt == 3))
                    nc.scalar.activation(dst[:, m, :], pt[:], AF.Relu,
                                         bias=bdown_sb[:, m:m + 1])

            # ---- S3/S4: q,k,v channel-major; norms over tokens per branch.
            # q is normalized on extraction; k's norm (x temperature) is kept
            # for the exp scale; v copied plain.
            q_sb = qsp.tile([128, 2, 512], BF, tag="q")
            k_sb = qsp.tile([128, 2, 512], BF, tag="k")
            v_sb = qsp.tile([128, 2, 512], BF, tag="v")
            # ssq2[:, g, 0:8] = q-norms, [:, g, 8:16] = k-norms; ln/exp ops
            # back-to-back so there is a single act-table site per tile.
            ssq2 = nrmp.tile([128, 2, 16], F32, tag="ssq2")
            lnt2 = nrmp.tile([128, 2, 16], F32, tag="lnt2")
            rn2 = nrmp.tile([128, 2, 16], F32, tag="rn2")
            sqscr = scr.tile([128, 2, 512], BF, tag="sqscr")

            # k first: its PSUM tiles are released by plain copies, so the
            # later q tiles can survive in the 3-deep ring until the rn2
            # chain completes (q-extract gates only the v matmuls).
            for g in range(2):
                pt = ps.tile([128, 512], F32, tag="ps")
                for kt in range(2):
                    nc.tensor.matmul(pt[:], wk_sb[:, kt, 128 * g:128 * (g + 1)],
                                     A[:, kt, :], start=(kt == 0), stop=(kt == 1))
                for bi, (b, off, w) in enumerate(bbs):
                    nc.scalar.activation(
                        sqscr[:, g, off:off + w], pt[:, off:off + w], AF.Square,
                        accum_out=ssq2[:, g, 8 + bi:9 + bi])
                nc.vector.tensor_copy(k_sb[:, g, :], pt[:])
            q_ps = []
            for g in range(2):
                pt = ps.tile([128, 512], F32, tag="ps")
                for kt in range(2):
                    nc.tensor.matmul(pt[:], wq_sb[:, kt, 128 * g:128 * (g + 1)],
                                     Bq[:, kt, :], start=(kt == 0), stop=(kt == 1))
                q_ps.append(pt)
                for bi, (b, off, w) in enumerate(bbs):
                    nc.scalar.activation(
                        sqscr[:, g, off:off + w], pt[:, off:off + w], AF.Square,
                        accum_out=ssq2[:, g, bi:bi + 1])

            # rn = (ssq + tiny)^-0.5 via exp(-0.5 ln(x+tiny)); k gets x temp
            # via bias=ln(temp). Single act-table site per tile.
            nc.scalar.activation(lnt2[:, :, 0:nb], ssq2[:, :, 0:nb],
                                 AF.Ln, bias=tiny_sb[:, 0:1])
            nc.scalar.activation(lnt2[:, :, 8:8 + nb], ssq2[:, :, 8:8 + nb],
                                 AF.Ln, bias=tiny_sb[:, 0:1])
            for g in range(2):
                nc.scalar.activation(rn2[:, g, 0:nb], lnt2[:, g, 0:nb],
                                     AF.Exp, scale=-0.5)
                nc.scalar.activation(rn2[:, g, 8:8 + nb], lnt2[:, g, 8:8 + nb],
                                     AF.Exp, scale=-0.5,
                                     bias=lntemp_sb[:, g:g + 1])
            rn_kt = rn2[:, :, 8:16]

            for g in range(2):
                for bi, (b, off, w) in enumerate(bbs):
                    nc.scalar.activation(
                        q_sb[:, g, off:off + w], q_ps[g][:, off:off + w],
                        AF.Copy, scale=rn2[:, g, bi:bi + 1])

            for g in range(2):
                pt = ps.tile([128, 512], F32, tag="ps")
                for kt in range(2):
                    nc.tensor.matmul(pt[:], wv_sb[:, kt, 128 * g:128 * (g + 1)],
                                     A[:, kt, :], start=(kt == 0), stop=(kt == 1))
                nc.vector.tensor_copy(v_sb[:, g, :], pt[:])

            return dict(x1t=x1t, r=r, A=A, q=q_sb, k=k_sb, v=v_sb, rn_kt=rn_kt)

        def emit_back(j, st):
            bbs = _bbs(j)
            nb = len(bbs)
            x1t, r, A = st["x1t"], st["r"], st["A"]
            q_sb, k_sb, v_sb, rn_kt = st["q"], st["k"], st["v"], st["rn_kt"]

            # ---- S5: PE-transpose q̂,k -> token-major qT,kT (bf16) ----
            qT = qtp.tile([128, 4, 256], BF, tag="qT")
            kT = qtp.tile([128, 4, 256], BF, tag="kT")
            for (dst, src) in ((qT, q_sb), (kT, k_sb)):
                for tb in range(4):
                    pt = ps.tile([128, 256], BF, tag="ps")
                    for g in range(2):
                        nc.tensor.matmul(
                            pt[:, 128 * g:128 * (g + 1)],
                            src[:, g, 128 * tb:128 * (tb + 1)], ident_sb[:],
                            is_transpose=True, start=(g == 0), stop=(g == 1))
                    nc.vector.tensor_copy(
                        dst[:, tb, :].bitcast(mybir.dt.uint32),
                        pt[:].bitcast(mybir.dt.uint32))

            # ---- S6-S8 per-branch attention: G^T -> mask-add -> exp into
            # block-diag slot -> S column + AV matmuls -> scaled extract.
            av = avp.tile([128, 2, 512], BF)
            S_sb = nrmp.tile([128, 2, 8], F32, tag="S_sb")
            R_t = nrmp.tile([128, 2, 8], F32, tag="R_t")
            for bi, (b, off, w) in enumerate(bbs):
                if j == 0:
                    chunks = [(off // 128, off % 128, 64)]
                else:
                    chunks = [(off // 128, 0, 128), (off // 128 + 1, 0, 128)]
                esc = escp.tile([128, 2, 128], F32, tag="esc")
                gps = [gpool.tile([128, 128], F32, tag="gps", name=f"gps{g_}")
                       for g_ in range(2)]
                scol = ps.tile([128, 2, 1], F32, tag="ps", name="scol")
                for g in range(2):
                    for ci, (tb, tpo, cw) in enumerate(chunks):
                        nc.tensor.matmul(
                            gps[g][:, :],
                            kT[tpo:tpo + cw, tb, 128 * g:128 * (g + 1)],
                            qT[tpo:tpo + cw, tb, 128 * g:128 * (g + 1)],
                            start=(ci == 0), stop=(ci == len(chunks) - 1))
                for g in range(2):
                    bsl = 2 * (bi % 2) + g
                    nc.vector.tensor_add(esc[:, g, :], gps[g][:], mask_sb[:])
                    nc.scalar.activation(bd[:, bsl, :], esc[:, g, :], AF.Exp,
                                         scale=rn_kt[:, g, bi:bi + 1])
                    nc.tensor.matmul(scol[:, g, :], bd[:, bsl, :],
                                     ones_sb[:, 0:1], start=True, stop=True)
                pav = psx.tile([128, 2, w], F32, tag="pav")
                for g in range(2):
                    bsl = 2 * (bi % 2) + g
                    nc.tensor.matmul(pav[:, g, :], bd[:, bsl, :],
                                     v_sb[:, g, off:off + w], start=True, stop=True)
                nc.scalar.activation(S_sb[:, :, bi:bi + 1], scol[:, :, :], AF.Copy)
                nc.vector.reciprocal(R_t[:, :, bi:bi + 1], S_sb[:, :, bi:bi + 1])
                for g in range(2):
                    nc.scalar.activation(
                        av[:, g, off:off + w], pav[:, g, :], AF.Copy,
                        scale=R_t[:, g, bi:bi + 1])

            # ---- S9: o1 = Wo@av + A (res1) ----
            o1 = o1p.tile([128, 2, 512], BF)
            for m in range(2):
                pt = ps.tile([128, 512], F32, tag="ps")
                for kt in range(2):
                    nc.tensor.matmul(pt[:], wo_sb[:, kt, 128 * m:128 * (m + 1)],
                                     av[:, kt, :], start=(kt == 0), stop=(kt == 1))
                nc.vector.tensor_add(o1[:, m, :], pt[:], A[:, m, :])

            # ---- S10: y = W_up^T o1 + b_up + r ----
            y = yp.tile([128, 4, 512], BF)
            for m in range(4):
                pt = ps.tile([128, 512], F32, tag="ps")
                for kt in range(2):
                    nc.tensor.matmul(pt[:], wup_sb[:, kt, 128 * m:128 * (m + 1)],
                                     o1[:, kt, :], start=(kt == 0), stop=(kt == 1))
                nc.vector.scalar_tensor_tensor(
                    y[:, m, :], in0=pt[:], scalar=bup_sb[:, m:m + 1],
                    in1=r[:, m, :], op0=OP.add, op1=OP.add)

            # ---- S11: pre = W_end^T y + b_end + t1 ; LN stats via ones-matmul
            pre = prep.tile([128, 4, 512], BF, tag="pre")
            p2 = prep.tile([128, 4, 512], BF, tag="p2")
            s1ps = psx.tile([128, 512], F32, tag="s1")
            s2ps = psx.tile([128, 512], F32, tag="s2")
            for m in range(4):
                pt = ps.tile([128, 512], F32, tag="ps")
                for kt in range(4):
                    nc.tensor.matmul(pt[:], wend_sb[:, kt, 128 * m:128 * (m + 1)],
                                     y[:, kt, :], start=(kt == 0), stop=(kt == 3))
                nc.vector.scalar_tensor_tensor(
                    pre[:, m, :], in0=pt[:], scalar=bend_sb[:, m:m + 1],
                    in1=x1t[:, m, :], op0=OP.add, op1=OP.add)
                nc.gpsimd.tensor_mul(p2[:, m, :], pre[:, m, :], pre[:, m, :])
                nc.tensor.matmul(s1ps[:], ones_sb[:], pre[:, m, :],
                                 start=(m == 0), stop=(m == 3))
                nc.tensor.matmul(s2ps[:], ones_sb[:], p2[:, m, :],
                                 start=(m == 0), stop=(m == 3))

            # ---- S12: mu / rstd (rows replicated); rstd = exp(-0.5 ln(var+eps))
            mu = statp.tile([128, 512], F32, tag="mu")
            nc.vector.tensor_scalar_mul(mu[:], in0=s1ps[:], scalar1=1.0 / DIM)
            mub = statp.tile([128, 512], BF, tag="mub")
            nc.vector.tensor_scalar_mul(mub[:], in0=s1ps[:], scalar1=1.0 / DIM)
            mu2 = statp.tile([128, 512], F32, tag="mu2")
            nc.vector.tensor_mul(mu2[:], mu[:], mu[:])
            var = statp.tile([128, 512], F32, tag="var")
            nc.vector.scalar_tensor_tensor(var[:], in0=s2ps[:], scalar=1.0 / DIM,
                                           in1=mu2[:], op0=OP.mult, op1=OP.subtract)
            lnv = statp.tile([128, 512], F32, tag="lnv")
            nc.scalar.activation(lnv[:], var[:], AF.Ln, bias=epsln_sb[:, 0:1])
            rstd = statp.tile([128, 512], BF, tag="rstd")
            nc.scalar.activation(rstd[:], lnv[:], AF.Exp, scale=-0.5)

            # ---- S13: out = (mu - pre)*rstd*(-gamma) + beta ----
            ot = outp.tile([128, 4, 512], BF)
            for m in range(4):
                d_m = scr.tile([128, 512], BF, tag="d_m")
                nc.vector.tensor_sub(d_m[:], mub[:], pre[:, m, :])
                e_m = scr.tile([128, 512], BF, tag="e_m")
                nc.vector.tensor_mul(e_m[:], d_m[:], rstd[:])
                nc.vector.tensor_scalar(
                    ot[:, m, :], in0=e_m[:], scalar1=gneg_sb[:, m:m + 1],
                    scalar2=beta_sb[:, m:m + 1], op0=OP.mult, op1=OP.add)

            # ---- S14: store (un-reorder tokens) ----
            for m in range(4):
                if j == 0:
                    nc.sync.dma_start(
                        outr[:, m, :, 0:64],
                        ot[:, m, :].rearrange("p (b t) -> p b t", b=8))
                else:
                    bs = slice(2 * (j - 1), 2 * j)
                    nc.sync.dma_start(
                        outr[:, m, bs, 64:320],
                        ot[:, m, :].rearrange("p (b t) -> p b t", b=2))

        prev = None
        order = [1, 2, 0, 3, 4]
        for j in order:
            ld = emit_loads(j)
            if j == order[0]:
                load_weights_early()
            st = emit_front(j, ld)
            if j == order[0]:
                load_weights_late()
            if prev is not None:
                emit_back(prev[0], prev[1])
            prev = (j, st)
        emit_back(prev[0], prev[1])

    nc.compile()
    return nc


# ---------------- host side ----------------
_CACHE = {}


def _get_runner():
    if "runner" in _CACHE:
        return _CACHE["runner"]
    import jax
    from jax.sharding import Mesh, PartitionSpec
    from jax.experimental.shard_map import shard_map
    from concourse.bass2jax import (
        _bass_exec_p, install_neuronx_cc_hook, partition_id_tensor)
    import concourse.mybir as mybir_

    nc = build_nc()
    install_neuronx_cc_hook()
    partition_name = nc.partition_id_tensor.name if nc.partition_id_tensor else None
    in_names, out_names, out_avals, zero_outs = [], [], [], []
    for alloc in nc.m.functions[0].allocations:
        if not isinstance(alloc, mybir_.MemoryLocationSet):
            continue
        name = alloc.memorylocations[0].name
        if alloc.kind == "ExternalInput":
            if name != partition_name:
                in_names.append(name)
        elif alloc.kind == "ExternalOutput":
            out_names.append(name)
            shape = tuple(alloc.tensor_shape)
            dtype = mybir_.dt.np(alloc.dtype)
            out_avals.append(jax.core.ShapedArray(shape, dtype))
            zero_outs.append(np.zeros(shape, dtype))
    n_params, n_outs = len(in_names), len(out_avals)
    all_in = list(in_names) + list(out_names)
    if partition_name is not None:
        all_in.append(partition_name)
    donate = tuple(range(n_params, n_params + n_outs))

    def _body(*args):
        operands = list(args)
        if partition_name is not None:
            operands.append(partition_id_tensor())
        return tuple(_bass_exec_p.bind(
            *operands, out_avals=tuple(out_avals), in_names=tuple(all_in),
            out_names=tuple(out_names), lowering_input_output_aliases=(),
            sim_require_finite=True, sim_require_nnan=True, nc=nc))

    devices = jax.devices()[:8]
    mesh = Mesh(np.asarray(devices), ("core",))
    fn = jax.jit(
        shard_map(_body, mesh=mesh,
                  in_specs=(PartitionSpec("core"),) * (n_params + n_outs),
                  out_specs=(PartitionSpec("core"),) * n_outs,
                  check_rep=False),
        donate_argnums=donate, keep_unused=True)
    _CACHE["runner"] = (fn, in_names, out_names, out_avals, zero_outs)
    return _CACHE["runner"]


def _prep_inputs(inputs):
    import ml_dtypes
    BFNP = ml_dtypes.bfloat16
    f = lambda a: np.ascontiguousarray(np.asarray(a), dtype=np.float32)
    bfc = lambda a: np.ascontiguousarray(np.asarray(a, dtype=np.float32).astype(BFNP))
    x1 = bfc(np.asarray(inputs["x1"], dtype=np.float32).reshape(64, DIM, NTOK))
    x2 = bfc(np.asarray(inputs["x2"], dtype=np.float32).reshape(64, DIM, NTOK))
    temp = f(inputs["temperature"]).reshape(HEADS)
    # lntemp_col[p, g] = ln(temperature[4*g + p//32])
    lntemp = np.log(np.maximum(temp, 1e-30)).astype(np.float32)
    lntemp_col = np.empty((128, 2), np.float32)
    for g in range(2):
        for hh in range(4):
            lntemp_col[32 * hh:32 * (hh + 1), g] = lntemp[4 * g + hh]
    shared = {
        "W_lin": bfc(inputs["W_lin"]), "W_down": bfc(inputs["W_down"]),
        "W_up": bfc(inputs["W_up"]),
        "WqT": bfc(np.asarray(inputs["Wq"], dtype=np.float32).T),
        "WkT": bfc(np.asarray(inputs["Wk"], dtype=np.float32).T),
        "WvT": bfc(np.asarray(inputs["Wv"], dtype=np.float32).T),
        "WoT": bfc(np.asarray(inputs["Wo"], dtype=np.float32).T),
        "W_end": bfc(inputs["W_end"]), "b_lin": f(inputs["b_lin"]),
        "b_down": f(inputs["b_down"]), "b_up": f(inputs["b_up"]),
        "b_end": f(inputs["b_end"]),
        "gneg": -f(inputs["gamma"]),
        "beta": f(inputs["beta"]), "lntemp_col": lntemp_col,
        "ident_in": np.eye(128).astype(BFNP),
    }
    in_maps = []
    for c in range(8):
        m = dict(shared)
        m["x1"] = np.ascontiguousarray(x1[8 * c:8 * (c + 1)])
        m["x2"] = np.ascontiguousarray(x2[8 * c:8 * (c + 1)])
        in_maps.append(m)
    return in_maps


def run_in_maps(in_maps):
    """Run the prebuilt executable on 8 cores; returns per-core out arrays."""
    import jax
    fn, in_names, out_names, out_avals, zero_outs = _get_runner()
    per_core = [[np.asarray(m[name]) for name in in_names] for m in in_maps]
    concat_in = [np.concatenate([per_core[c][i] for c in range(8)], axis=0)
                 for i in range(len(in_names))]
    concat_zeros = [np.zeros((8 * z.shape[0], *z.shape[1:]), z.dtype)
                    for z in zero_outs]
    out = fn(*concat_in, *concat_zeros)
    jax.block_until_ready(out)
    oi = out_names.index("out")
    arr = np.asarray(out[oi]).reshape(8, *out_avals[oi].shape)
    return arr


def kernel(**inputs):
    in_maps = _prep_inputs(inputs)
    arr = run_in_maps(in_maps)  # [8, 8, 512, 320] bf16
    full = arr.astype(np.float32).reshape(64, DIM, NTOK).reshape(64, DIM, 16, 20)
    return full


if __name__ == "__main__":
    rng = np.random.default_rng(0)
    ins = {
        "x1": rng.standard_normal((64, 512, 16, 20), dtype=np.float32),
        "x2": rng.standard_normal((64, 512, 16, 20), dtype=np.float32),
    }
    s = 0.02
    for nm, shape in [("W_lin", (512, 1024)), ("W_down", (512, 256)),
                      ("W_up", (256, 512)), ("Wq", (256, 256)),
                      ("Wk", (256, 256)), ("Wv", (256, 256)),
                      ("Wo", (256, 256)), ("W_end", (512, 512))]:
        ins[nm] = (rng.standard_normal(shape) * s).astype(np.float32)
    for nm, n in [("b_lin", 1024), ("b_down", 256), ("b_up", 512),
                  ("b_end", 512)]:
        ins[nm] = np.zeros(n, np.float32)
    ins["gamma"] = np.ones(512, np.float32)
    ins["beta"] = np.zeros(512, np.float32)
    ins["temperature"] = np.ones((8, 1, 1), np.float32)
    out = kernel(**ins)
    print("kernel ran, out shape", out.shape, "mean", float(np.abs(out).mean()))
